# revision 15
# baseline (speedup 1.0000x reference)
"""Mixtral decoder layer (attention + top-2 MoE) on 8 TRN2 NeuronCores.

Sharding:
  - Attention: batch (2) x head-group (4): core i handles batch i//4, q-heads
    [4*(i%4), 4*(i%4)+4)  (GQA: kv heads 2*(i%4), 2*(i%4)+1).
    Wo partials are ReduceScattered within each quad -> core i owns global
    token slice [512*i, 512*(i+1)).
  - MoE: expert-parallel, expert e = core i.  h2 (bf16) + router logits (fp32)
    are AllGathered; each core routes/compacts all 4096 tokens for its expert
    (capacity C), runs the FFN, scatters back by token, and a ReduceScatter
    over all 8 cores sums expert contributions back to the owning token slice.
"""

import numpy as np
import ml_dtypes

import concourse.bass as bass
import concourse.mybir as mybir
import concourse.tile as tile
from concourse import bacc
from concourse.bass import ds, ts
from concourse.bass_utils import run_bass_kernel_spmd
from concourse.masks import make_identity, make_causal_mask, make_upper_triangular

F32 = mybir.dt.float32
BF16 = mybir.dt.bfloat16
I32 = mybir.dt.int32
AF = mybir.ActivationFunctionType
OP = mybir.AluOpType
AX = mybir.AxisListType

P = 128
B, S, H = 2, 2048, 1024
NH, NKV, HD = 16, 8, 64
E, TOPK, I = 8, 2, 3584
T = B * S            # 4096 tokens
TS = T // 8          # 512 tokens owned per core
C = 1280             # expert token capacity (avg load is 1024)
EPS = 1e-6
NEG = -1e30

KSUB = H // P        # 8 contraction subtiles of hidden dim
ISUB = I // P        # 28 subtiles of intermediate dim
NT_B = S // P        # 16 token tiles per batch
NT_S = TS // P       # 4 token tiles per owned slice
NT_T = T // P        # 32 token tiles globally
NC_C = C // P        # 10 compact tiles


def build_nc():
    nc = bacc.Bacc("TRN2", target_bir_lowering=False, debug=False)
    nc.num_devices = 8

    x_full = nc.declare_dram_parameter("x_full", [S, H], F32, isOutput=False)
    x_resid = nc.declare_dram_parameter("x_resid", [TS, H], F32, isOutput=False)
    wq = nc.declare_dram_parameter("wq", [H, 4 * HD], BF16, isOutput=False)
    wk = nc.declare_dram_parameter("wk", [H, 2 * HD], BF16, isOutput=False)
    wv = nc.declare_dram_parameter("wv", [H, 2 * HD], BF16, isOutput=False)
    wo_pad = nc.declare_dram_parameter("wo_pad", [4 * P, H], BF16, isOutput=False)
    cosT = nc.declare_dram_parameter("cosT", [HD, S], F32, isOutput=False)
    sinTs = nc.declare_dram_parameter("sinTs", [HD, S], F32, isOutput=False)
    ln1w = nc.declare_dram_parameter("ln1w", [P, H], F32, isOutput=False)
    ln2w = nc.declare_dram_parameter("ln2w", [P, H], F32, isOutput=False)
    gate = nc.declare_dram_parameter("gate", [H, E], F32, isOutput=False)
    w1 = nc.declare_dram_parameter("w1", [H, I], BF16, isOutput=False)
    w3 = nc.declare_dram_parameter("w3", [H, I], BF16, isOutput=False)
    w2 = nc.declare_dram_parameter("w2", [I, H], BF16, isOutput=False)
    myid = nc.declare_dram_parameter("myid", [P, 1], F32, isOutput=False)
    out_p = nc.declare_dram_parameter("out", [TS, H], F32, isOutput=True)

    RG_QUAD = [[0, 1, 2, 3], [4, 5, 6, 7]]
    RG_ALL = [[0, 1, 2, 3, 4, 5, 6, 7]]

    import contextlib
    with tile.TileContext(nc) as tc, contextlib.ExitStack() as gctx:
        dram = gctx.enter_context(tc.tile_pool(name="dram", bufs=1, space="DRAM"))
        rs_in = dram.tile([S, H], F32, name="rs_in")
        rs_out = dram.tile([TS, H], F32, name="rs_out")
        ag_in = dram.tile([TS, H], BF16, name="ag_in")
        h2_all = dram.tile([T, H], BF16, addr_space="Shared", name="h2_all")
        lg_in = dram.tile([TS, E], F32, name="lg_in")
        lg_all = dram.tile([T, E], F32, addr_space="Shared", name="lg_all")
        inv_d = dram.tile([C, 1], I32, name="inv_d")
        y_d = dram.tile([C, H], F32, name="y_d")
        rs2_in = dram.tile([T, H], BF16, name="rs2_in")
        rs2_out = dram.tile([TS, H], BF16, name="rs2_out")

        # ---------------- constants ----------------
        const = gctx.enter_context(tc.tile_pool(name="const", bufs=1))
        ident_bf = const.tile([P, P], BF16, name="ident_bf")
        make_identity(nc, ident_bf[:])
        ident_f = const.tile([P, P], F32, name="ident_f")
        make_identity(nc, ident_f[:])
        causal = const.tile([P, P], F32, name="causal")
        make_causal_mask(nc, causal[:], mask_val=NEG)
        strictU = const.tile([P, P], F32, name="strictU")
        make_upper_triangular(nc, strictU[:], val=1.0, diag=False)
        ln1_sb = const.tile([P, H], F32, name="ln1_sb")
        nc.sync.dma_start(ln1_sb[:], ln1w[:, :])
        ln2_sb = const.tile([P, H], F32, name="ln2_sb")
        nc.sync.dma_start(ln2_sb[:], ln2w[:, :])
        myid_sb = const.tile([P, 1], F32, name="myid_sb")
        nc.sync.dma_start(myid_sb[:], myid[:, :])
        epsb = const.tile([P, 1], F32, name="epsb")
        nc.vector.memset(epsb[:], EPS)
        zerob = const.tile([P, 1], F32, name="zerob")
        nc.vector.memset(zerob[:], 0.0)

        # persistent across attention + moe phases
        pers = gctx.enter_context(tc.tile_pool(name="pers", bufs=1))
        x2_sb = pers.tile([P, NT_S, H], F32, name="x2_sb")     # 2 MB

        # ================= attention =================
        actx = contextlib.ExitStack()
        sb = actx.enter_context(tc.tile_pool(name="attn_sb", bufs=2))
        sbs = actx.enter_context(tc.tile_pool(name="attn_sbs", bufs=4))
        sbr = actx.enter_context(tc.tile_pool(name="attn_sbr", bufs=1))
        sb1 = actx.enter_context(tc.tile_pool(name="attn_sb1", bufs=1))
        psT = actx.enter_context(tc.tile_pool(name="attn_psT", bufs=2, space="PSUM"))
        psA = actx.enter_context(tc.tile_pool(name="attn_psA", bufs=2, space="PSUM"))
        psV = actx.enter_context(tc.tile_pool(name="attn_psV", bufs=2, space="PSUM"))
        hT = sb1.tile([P, KSUB, S], BF16, name="hT")           # 4 MB

        # ---- rmsnorm(x) -> hT (feature-major, bf16) ----
        for tt in range(NT_B):
            xt = sb.tile([P, H], F32, tag="xt")
            nc.sync.dma_start(xt[:], x_full[ts(tt, P), :])
            sq = sbr.tile([P, H], F32, tag="sq")
            ssum = sbs.tile([P, 1], F32, tag="ssum")
            nc.scalar.activation(sq[:], xt[:], AF.Square, bias=zerob[:, 0:1], accum_out=ssum[:])
            rms = sbs.tile([P, 1], F32, tag="rms")
            nc.scalar.activation(rms[:], ssum[:], AF.Sqrt, scale=1.0 / H, bias=epsb[:, 0:1])
            rinv = sbs.tile([P, 1], F32, tag="rinv")
            nc.vector.reciprocal(rinv[:], rms[:])
            nc.vector.tensor_scalar_mul(xt[:], xt[:], rinv[:, 0:1])
            hb = sb.tile([P, H], BF16, tag="hb")
            nc.vector.tensor_mul(hb[:], xt[:], ln1_sb[:])
            for ks in range(KSUB):
                ptr = psT.tile([P, P], BF16, tag="ptr")
                nc.tensor.transpose(ptr[:], hb[:, ts(ks, P)], ident_bf[:])
                nc.any.tensor_copy(hT[:, ks, ts(tt, P)], ptr[:])

        # ---- weights for projections ----
        wq_sb = sb1.tile([P, KSUB, 4 * HD], BF16, name="wq_sb")
        nc.sync.dma_start(wq_sb[:], wq.ap().rearrange("(ks p) n -> p ks n", p=P))
        wk_sb = sb1.tile([P, KSUB, 2 * HD], BF16, name="wk_sb")
        nc.sync.dma_start(wk_sb[:], wk.ap().rearrange("(ks p) n -> p ks n", p=P))
        wv_sb = sb1.tile([P, KSUB, 2 * HD], BF16, name="wv_sb")
        nc.sync.dma_start(wv_sb[:], wv.ap().rearrange("(ks p) n -> p ks n", p=P))
        wo_sb = sb1.tile([P, 4, H], BF16, name="wo_sb")
        nc.sync.dma_start(wo_sb[:], wo_pad.ap().rearrange("(h p) n -> p h n", p=P))
        cos_sb = sb1.tile([HD, S], F32, name="cos_sb")
        nc.sync.dma_start(cos_sb[:], cosT[:, :])
        sin_sb = sb1.tile([HD, S], F32, name="sin_sb")
        nc.sync.dma_start(sin_sb[:], sinTs[:, :])

        # ---- q/k projections + rope (per head, zero-padded to 128 parts) ----
        qp = []   # 4 q heads: [P, S] bf16, parts [0:64] live
        kp = []   # 2 kv heads
        for h in range(4):
            q_h = sb1.tile([P, S], BF16, name=f"q_{h}")
            nc.vector.memset(q_h[64:128, :], 0.0)
            qp.append(q_h)
        for h in range(2):
            k_h = sb1.tile([P, S], BF16, name=f"k_{h}")
            nc.vector.memset(k_h[64:128, :], 0.0)
            kp.append(k_h)

        def proj_rope(dst, w_sb, col0, scale):
            """project head (cols [col0, col0+64) of w_sb) for all S tokens,
            apply rope, write bf16 into dst[0:64, :]."""
            hf = sbr.tile([HD, S], F32, tag="ropef")
            for tch in range(S // 512):
                ps = psA.tile([P, 512], F32, tag="psA")
                for ks in range(KSUB):
                    nc.tensor.matmul(
                        ps[:HD, :], lhsT=w_sb[:, ks, col0:col0 + HD],
                        rhs=hT[:, ks, ts(tch, 512)],
                        start=(ks == 0), stop=(ks == KSUB - 1))
                nc.any.tensor_copy(hf[:, ts(tch, 512)], ps[:HD, :])
            swp = sbr.tile([HD, S], F32, tag="ropeswp")
            nc.sync.dma_start(swp[0:32, :], hf[32:64, :])
            nc.sync.dma_start(swp[32:64, :], hf[0:32, :])
            nc.vector.tensor_mul(hf[:], hf[:], cos_sb[:])
            nc.vector.tensor_mul(swp[:], swp[:], sin_sb[:])
            nc.vector.tensor_add(hf[:], hf[:], swp[:])
            nc.vector.tensor_scalar_mul(dst[0:64, :], hf[:], scale)

        for h in range(4):
            proj_rope(qp[h], wq_sb, h * HD, 0.125)  # 1/sqrt(64) folded into q
        for h in range(2):
            proj_rope(kp[h], wk_sb, h * HD, 1.0)

        # ---- v projection, token-major [P, NT_B, 128] (two heads on free) ----
        v_sb = sb1.tile([P, NT_B, 2 * HD], BF16, name="v_sb")
        for tt in range(NT_B):
            ps = psA.tile([P, 512], F32, tag="psA")
            for ks in range(KSUB):
                nc.tensor.matmul(
                    ps[:, 0:2 * HD], lhsT=hT[:, ks, ts(tt, P)], rhs=wv_sb[:, ks, :],
                    start=(ks == 0), stop=(ks == KSUB - 1))
            nc.any.tensor_copy(v_sb[:, tt, :], ps[:, 0:2 * HD])

        # ---- attention core ----
        attnT = []
        for h in range(4):
            a_h = sb1.tile([P, S], BF16, name=f"attnT_{h}")
            nc.vector.memset(a_h[64:128, :], 0.0)
            attnT.append(a_h)

        for h in range(4):
            k_h = kp[h // 2]
            v_col0 = (h // 2) * HD
            for qt in range(NT_B):
                nkt = qt + 1                      # causal: key tiles 0..qt
                nkeys = nkt * P
                s_sb = sb.tile([P, S], F32, tag="s_sb")
                for ch in range((nkeys + 511) // 512):
                    cw = min(512, nkeys - ch * 512)
                    ps = psA.tile([P, 512], F32, tag="psA")
                    nc.tensor.matmul(
                        ps[:, :cw], lhsT=qp[h][:, ts(qt, P)],
                        rhs=k_h[:, ds(ch * 512, cw)], start=True, stop=True)
                    nc.any.tensor_copy(s_sb[:, ds(ch * 512, cw)], ps[:, :cw])
                # causal mask on the diagonal tile
                nc.vector.tensor_add(s_sb[:, ts(qt, P)], s_sb[:, ts(qt, P)], causal[:])
                mx = sbs.tile([P, 1], F32, tag="mx")
                nc.vector.reduce_max(mx[:], s_sb[:, 0:nkeys], axis=AX.X)
                nmx = sbs.tile([P, 1], F32, tag="nmx")
                nc.vector.tensor_scalar_mul(nmx[:], mx[:], -1.0)
                p_sb = sb.tile([P, S], BF16, tag="p_sb")
                esum = sbs.tile([P, 1], F32, tag="esum")
                nc.scalar.activation(p_sb[:, 0:nkeys], s_sb[:, 0:nkeys], AF.Exp,
                                     bias=nmx[:, 0:1], accum_out=esum[:])
                rsum = sbs.tile([P, 1], F32, tag="rsum")
                nc.vector.reciprocal(rsum[:], esum[:])
                nc.vector.tensor_scalar_mul(p_sb[:, 0:nkeys], p_sb[:, 0:nkeys],
                                            rsum[:, 0:1])
                av = psV.tile([HD, P], F32, tag="av")
                for kt in range(nkt):
                    ptp = psT.tile([P, P], BF16, tag="ptp")
                    nc.tensor.transpose(ptp[:], p_sb[:, ts(kt, P)], ident_bf[:])
                    pT = sbs.tile([P, P], BF16, tag="pT")
                    nc.any.tensor_copy(pT[:], ptp[:])
                    nc.tensor.matmul(
                        av[:], lhsT=v_sb[:, kt, v_col0:v_col0 + HD], rhs=pT[:],
                        start=(kt == 0), stop=(kt == nkt - 1))
                nc.any.tensor_copy(attnT[h][0:HD, ts(qt, P)], av[:])

        # ---- wo partial -> rs_in ----
        for tt in range(NT_B):
            ro = sb.tile([P, H], F32, tag="ro")
            for nh in range(2):
                ps = psA.tile([P, 512], F32, tag="psA")
                for h in range(4):
                    nc.tensor.matmul(
                        ps[:], lhsT=attnT[h][:, ts(tt, P)],
                        rhs=wo_sb[:, h, ts(nh, 512)],
                        start=(h == 0), stop=(h == 3))
                nc.any.tensor_copy(ro[:, ts(nh, 512)], ps[:])
            nc.sync.dma_start(rs_in[ts(tt, P), :], ro[:])

        actx.close()

        # ================= quad reduce-scatter =================
        nc.gpsimd.collective_compute(
            "ReduceScatter", OP.add, replica_groups=RG_QUAD,
            ins=[rs_in[:, :].opt()], outs=[rs_out[:, :].opt()])

        # ================= router on owned slice =================
        rctx = contextlib.ExitStack()
        rb = rctx.enter_context(tc.tile_pool(name="rout_sb", bufs=3))
        rb1 = rctx.enter_context(tc.tile_pool(name="rout_sb1", bufs=1))
        psR = rctx.enter_context(tc.tile_pool(name="rout_ps", bufs=2, space="PSUM"))

        gate_sb = rb1.tile([P, KSUB, E], F32, name="gate_sb")
        nc.sync.dma_start(gate_sb[:], gate.ap().rearrange("(ks p) e -> p ks e", p=P))
        h2T = rb1.tile([P, KSUB, TS], F32, name="h2T")

        for tt in range(NT_S):
            rsx = rb.tile([P, H], F32, tag="rsx")
            nc.sync.dma_start(rsx[:], rs_out[ts(tt, P), :])
            xres = rb.tile([P, H], F32, tag="xres")
            nc.sync.dma_start(xres[:], x_resid[ts(tt, P), :])
            nc.vector.tensor_add(x2_sb[:, tt, :], rsx[:], xres[:])
            # rmsnorm
            sq = rb.tile([P, H], F32, tag="sq2")
            ssum = rb.tile([P, 1], F32, tag="ssum2")
            nc.scalar.activation(sq[:], x2_sb[:, tt, :], AF.Square, bias=zerob[:, 0:1], accum_out=ssum[:])
            rms = rb.tile([P, 1], F32, tag="rms2")
            nc.scalar.activation(rms[:], ssum[:], AF.Sqrt, scale=1.0 / H, bias=epsb[:, 0:1])
            rinv = rb.tile([P, 1], F32, tag="rinv2")
            nc.vector.reciprocal(rinv[:], rms[:])
            h2f = rb.tile([P, H], F32, tag="h2f")
            nc.vector.tensor_scalar_mul(h2f[:], x2_sb[:, tt, :], rinv[:, 0:1])
            nc.vector.tensor_mul(h2f[:], h2f[:], ln2_sb[:])
            h2b = rb.tile([P, H], BF16, tag="h2b")
            nc.vector.tensor_copy(h2b[:], h2f[:])
            nc.sync.dma_start(ag_in[ts(tt, P), :], h2b[:])
            for ks in range(KSUB):
                ptr = psR.tile([P, P], F32, tag="ptr2")
                nc.tensor.transpose(ptr[:], h2f[:, ts(ks, P)], ident_f[:])
                nc.any.tensor_copy(h2T[:, ks, ts(tt, P)], ptr[:])
        # logits (token-major) in fp32
        for tt in range(NT_S):
            psl = psR.tile([P, E], F32, tag="psl")
            for ks in range(KSUB):
                nc.tensor.matmul(
                    psl[:], lhsT=h2T[:, ks, ts(tt, P)], rhs=gate_sb[:, ks, :],
                    start=(ks == 0), stop=(ks == KSUB - 1))
            lgt = rb.tile([P, E], F32, tag="lgt")
            nc.any.tensor_copy(lgt[:], psl[:])
            nc.sync.dma_start(lg_in[ts(tt, P), :], lgt[:])
        rctx.close()

        # ================= allgathers =================
        nc.gpsimd.collective_compute(
            "AllGather", OP.bypass, replica_groups=RG_ALL,
            ins=[ag_in[:, :].opt()], outs=[h2_all[:, :].opt()])
        nc.gpsimd.collective_compute(
            "AllGather", OP.bypass, replica_groups=RG_ALL,
            ins=[lg_in[:, :].opt()], outs=[lg_all[:, :].opt()])

        # ================= routing for all tokens =================
        mctx = contextlib.ExitStack()
        mb = mctx.enter_context(tc.tile_pool(name="moe_sb", bufs=3))
        mb1 = mctx.enter_context(tc.tile_pool(name="moe_sb1", bufs=1))
        psM = mctx.enter_context(tc.tile_pool(name="moe_ps", bufs=2, space="PSUM"))
        psM1 = mctx.enter_context(tc.tile_pool(name="moe_ps1", bufs=1, space="PSUM"))

        NJ = NT_T  # 32 columns, token t = j*128 + p
        lg = mb1.tile([P, NJ, E], F32, name="lg")
        nc.sync.dma_start(lg[:], lg_all[:, :].rearrange("(j p) e -> p j e", p=P))
        iot = mb1.tile([P, E], I32, name="iot")
        nc.gpsimd.iota(iot[:], pattern=[[1, E]], base=0, channel_multiplier=0)
        iotf = mb1.tile([P, E], F32, name="iotf")
        nc.vector.tensor_copy(iotf[:], iot[:])
        onehot = mb1.tile([P, E], F32, name="onehot")
        nc.vector.tensor_tensor(onehot[:], iotf[:],
                                myid_sb[:, 0:1].to_broadcast([P, E]), op=OP.is_equal)

        m1 = mb1.tile([P, NJ], F32, name="m1")
        nc.vector.reduce_max(m1[:], lg[:], axis=AX.X)
        mask1 = mb1.tile([P, NJ, E], F32, name="mask1")
        nc.vector.tensor_tensor(mask1[:], lg[:],
                                m1[:, :, None].to_broadcast([P, NJ, E]),
                                op=OP.is_ge)
        lg2 = mb1.tile([P, NJ, E], F32, name="lg2")
        nc.vector.tensor_scalar_mul(lg2[:], mask1[:], NEG)
        nc.vector.tensor_add(lg2[:], lg2[:], lg[:])
        m2 = mb1.tile([P, NJ], F32, name="m2")
        nc.vector.reduce_max(m2[:], lg2[:], axis=AX.X)
        mask2 = mb1.tile([P, NJ, E], F32, name="mask2")
        nc.vector.tensor_tensor(mask2[:], lg2[:],
                                m2[:, :, None].to_broadcast([P, NJ, E]),
                                op=OP.is_ge)
        # sel_k = <mask_k, onehot>  (0/1), k=1,2
        sel1 = mb1.tile([P, NJ], F32, name="sel1")
        tmp3 = mb1.tile([P, NJ, E], F32, name="tmp3")
        nc.vector.tensor_mul(tmp3[:], mask1[:],
                             onehot[:, None, :].to_broadcast([P, NJ, E]))
        nc.vector.reduce_sum(sel1[:], tmp3[:], axis=AX.X)
        sel2 = mb1.tile([P, NJ], F32, name="sel2")
        nc.vector.tensor_mul(tmp3[:], mask2[:],
                             onehot[:, None, :].to_broadcast([P, NJ, E]))
        nc.vector.reduce_sum(sel2[:], tmp3[:], axis=AX.X)
        # top-2 weights: w1 = 1/(1+exp(m2-m1)), w2 = 1-w1
        dlt = mb1.tile([P, NJ], F32, name="dlt")
        nc.vector.tensor_sub(dlt[:], m2[:], m1[:])
        edl = mb1.tile([P, NJ], F32, name="edl")
        nc.scalar.activation(edl[:], dlt[:], AF.Exp, bias=zerob[:, 0:1])
        nc.vector.tensor_scalar_add(edl[:], edl[:], 1.0)
        wt1 = mb1.tile([P, NJ], F32, name="wt1")
        nc.vector.reciprocal(wt1[:], edl[:])
        wt2 = mb1.tile([P, NJ], F32, name="wt2")
        nc.scalar.activation(wt2[:], wt1[:], AF.Copy, bias=1.0, scale=-1.0)
        we = mb1.tile([P, NJ], F32, name="we")
        nc.vector.tensor_mul(we[:], sel1[:], wt1[:])
        tmp2 = mb1.tile([P, NJ], F32, name="tmp2")
        nc.vector.tensor_mul(tmp2[:], sel2[:], wt2[:])
        nc.vector.tensor_add(we[:], we[:], tmp2[:])
        sel = mb1.tile([P, NJ], F32, name="sel")
        nc.vector.tensor_add(sel[:], sel1[:], sel2[:])

        # compaction: slot = prefix_p + exclusive_rank_j, else C-1
        zer = mb1.tile([P, NJ], F32, name="zer")
        nc.vector.memset(zer[:], 0.0)
        csum = mb1.tile([P, NJ], F32, name="csum")
        nc.vector.tensor_tensor_scan(csum[:], sel[:], zer[:], 0.0,
                                     op0=OP.add, op1=OP.add)
        rank = mb1.tile([P, NJ], F32, name="rank")
        nc.vector.tensor_sub(rank[:], csum[:], sel[:])
        counts = mb1.tile([P, 1], F32, name="counts")
        nc.vector.tensor_copy(counts[:], csum[:, NJ - 1:NJ])
        pfx = psM1.tile([P, 1], F32, tag="pfx")
        nc.tensor.matmul(pfx[:], lhsT=strictU[:], rhs=counts[:],
                         start=True, stop=True)
        pfx_sb = mb1.tile([P, 1], F32, name="pfx_sb")
        nc.any.tensor_copy(pfx_sb[:], pfx[:])
        slot = mb1.tile([P, NJ], F32, name="slot")
        nc.vector.tensor_add(slot[:], rank[:],
                             pfx_sb[:, 0:1].to_broadcast([P, NJ]))
        # unselected -> C-1 ; clamp to [0, C-1]
        nc.vector.tensor_scalar_add(slot[:], slot[:], float(-(C - 1)))
        nc.vector.tensor_mul(slot[:], slot[:], sel[:])
        nc.vector.tensor_scalar_add(slot[:], slot[:], float(C - 1))
        nc.vector.tensor_scalar_min(slot[:], slot[:], float(C - 1))
        slot_i = mb1.tile([P, NJ], I32, name="slot_i")
        nc.vector.tensor_copy(slot_i[:], slot[:])

        # inverse permutation: inv_d[slot[t]] = t
        zeri = mb.tile([P, NC_C], I32, tag="zeri")
        nc.vector.memset(zeri[:], 0)
        nc.sync.dma_start(inv_d[:, :].rearrange("(j p) one -> p (j one)", p=P),
                          zeri[:])
        tok = mb1.tile([P, NJ], I32, name="tok")
        nc.gpsimd.iota(tok[:], pattern=[[P, NJ]], base=0, channel_multiplier=1)
        for j in range(NJ):
            nc.gpsimd.indirect_dma_start(
                out=inv_d[:, :],
                out_offset=bass.IndirectOffsetOnAxis(ap=slot_i[:, j:j + 1], axis=0),
                in_=tok[:, j:j + 1], in_offset=None)

        # gather h2 rows for this expert -> transpose to feature-major
        zctx = contextlib.ExitStack()
        zpool = zctx.enter_context(tc.tile_pool(name="zpool", bufs=1))
        hselT = zpool.tile([P, KSUB, C], BF16, name="hselT")
        for ct in range(NC_C):
            idx = mb.tile([P, 1], I32, tag="idx")
            nc.sync.dma_start(idx[:], inv_d[ts(ct, P), :])
            hrow = mb.tile([P, H], BF16, tag="hrow")
            nc.gpsimd.indirect_dma_start(
                out=hrow[:], out_offset=None,
                in_=h2_all[:, :],
                in_offset=bass.IndirectOffsetOnAxis(ap=idx[:, 0:1], axis=0))
            for ks in range(KSUB):
                ptr = psM1.tile([P, P], BF16, tag="ptr3")
                nc.tensor.transpose(ptr[:], hrow[:, ts(ks, P)], ident_bf[:])
                nc.any.tensor_copy(hselT[:, ks, ts(ct, P)], ptr[:])

        # ---- expert FFN ----
        z_sb = mb1.tile([P, ISUB, C], BF16, name="z_sb")       # 9.2 MB
        CCH = [(0, 512), (512, 512), (1024, C - 1024)]
        wpool = zctx.enter_context(tc.tile_pool(name="wpool", bufs=2))
        for mc in range(ISUB // 4):                 # 7 chunks of 4 I-subtiles
            w1_sb = wpool.tile([P, KSUB, 512], BF16, tag="w1_sb")
            nc.sync.dma_start(
                w1_sb[:],
                w1.ap().rearrange("(ks p) i -> p ks i", p=P)[:, :, ts(mc, 512)])
            w3_sb = wpool.tile([P, KSUB, 512], BF16, tag="w3_sb")
            nc.sync.dma_start(
                w3_sb[:],
                w3.ap().rearrange("(ks p) i -> p ks i", p=P)[:, :, ts(mc, 512)])
            for mi in range(4):
                for c0, cw in CCH:
                    ps1 = psM.tile([P, 512], F32, tag="ps1")
                    ps3 = psM.tile([P, 512], F32, tag="ps3")
                    for ks in range(KSUB):
                        nc.tensor.matmul(
                            ps1[:, :cw], lhsT=w1_sb[:, ks, ts(mi, P)],
                            rhs=hselT[:, ks, ds(c0, cw)],
                            start=(ks == 0), stop=(ks == KSUB - 1))
                    for ks in range(KSUB):
                        nc.tensor.matmul(
                            ps3[:, :cw], lhsT=w3_sb[:, ks, ts(mi, P)],
                            rhs=hselT[:, ks, ds(c0, cw)],
                            start=(ks == 0), stop=(ks == KSUB - 1))
                    zs = mb.tile([P, 512], BF16, tag="zs")
                    nc.scalar.activation(zs[:, :cw], ps1[:, :cw], AF.Silu, bias=zerob[:, 0:1])
                    nc.vector.tensor_mul(z_sb[:, mc * 4 + mi, ds(c0, cw)],
                                         zs[:, :cw], ps3[:, :cw])
        zctx.close()
        w2ctx = contextlib.ExitStack()
        w2pool = w2ctx.enter_context(tc.tile_pool(name="w2pool", bufs=1))
        w2_sb = w2pool.tile([P, ISUB, H], BF16, name="w2_sb")  # 7.2 MB
        nc.sync.dma_start(w2_sb[:], w2.ap().rearrange("(ks p) n -> p ks n", p=P))
        for tc_i in range(NC_C):
            yo = mb.tile([P, H], F32, tag="yo")
            for nh in range(2):
                psy = psM.tile([P, 512], F32, tag="psy")
                for ks in range(ISUB):
                    nc.tensor.matmul(
                        psy[:], lhsT=z_sb[:, ks, ts(tc_i, P)],
                        rhs=w2_sb[:, ks, ts(nh, 512)],
                        start=(ks == 0), stop=(ks == ISUB - 1))
                nc.any.tensor_copy(yo[:, ts(nh, 512)], psy[:])
            nc.sync.dma_start(y_d[ts(tc_i, P), :], yo[:])
        w2ctx.close()

        # ---- gather back by token, weight, emit rs2_in ----
        for j in range(NJ):
            yg = mb.tile([P, H], F32, tag="yg")
            nc.gpsimd.indirect_dma_start(
                out=yg[:], out_offset=None,
                in_=y_d[:, :],
                in_offset=bass.IndirectOffsetOnAxis(ap=slot_i[:, j:j + 1], axis=0))
            mo = mb.tile([P, H], BF16, tag="mo")
            nc.vector.tensor_scalar_mul(mo[:], yg[:], we[:, j:j + 1])
            nc.sync.dma_start(rs2_in[ts(j, P), :], mo[:])
        mctx.close()

        # ================= final reduce-scatter + residual =================
        nc.gpsimd.collective_compute(
            "ReduceScatter", OP.add, replica_groups=RG_ALL,
            ins=[rs2_in[:, :].opt()], outs=[rs2_out[:, :].opt()])

        fctx = contextlib.ExitStack()
        fb = fctx.enter_context(tc.tile_pool(name="fin_sb", bufs=3))
        for tt in range(NT_S):
            mg = fb.tile([P, H], BF16, tag="mg")
            nc.sync.dma_start(mg[:], rs2_out[ts(tt, P), :])
            oo = fb.tile([P, H], F32, tag="oo")
            nc.vector.tensor_add(oo[:], x2_sb[:, tt, :], mg[:])
            nc.sync.dma_start(out_p[ts(tt, P), :], oo[:])
        fctx.close()

    nc.compile()
    return nc


_NC_CACHE = None


def _get_nc():
    global _NC_CACHE
    if _NC_CACHE is None:
        _NC_CACHE = build_nc()
    return _NC_CACHE


def _prep_inputs(inputs):
    bf16 = ml_dtypes.bfloat16
    f32 = np.float32
    x = np.asarray(inputs["hidden_states"], f32)          # [B, S, H]
    wq = np.asarray(inputs["wq"], f32)
    wk = np.asarray(inputs["wk"], f32)
    wv = np.asarray(inputs["wv"], f32)
    wo = np.asarray(inputs["wo"], f32)
    gate = np.ascontiguousarray(np.asarray(inputs["gate_w"], f32))
    w1 = np.asarray(inputs["w1"], f32)
    w2 = np.asarray(inputs["w2"], f32)
    w3 = np.asarray(inputs["w3"], f32)
    ln1 = np.asarray(inputs["ln1_w"], f32)
    ln2 = np.asarray(inputs["ln2_w"], f32)
    cos = np.asarray(inputs["rope_cos"], f32)             # [S, HD]
    sin = np.asarray(inputs["rope_sin"], f32)

    cosT = np.ascontiguousarray(cos.T)                    # [HD, S]
    sinT = np.ascontiguousarray(sin.T).copy()
    sinT[:HD // 2] *= -1.0                                # rotate_half sign
    ln1_t = np.ascontiguousarray(np.broadcast_to(ln1, (P, H)))
    ln2_t = np.ascontiguousarray(np.broadcast_to(ln2, (P, H)))

    in_maps = []
    for i in range(8):
        b, g = i // 4, i % 4
        wo_pad = np.zeros((4, P, H), f32)
        for h in range(4):
            wo_pad[h, :HD] = wo[g * 4 * HD + h * HD: g * 4 * HD + (h + 1) * HD]
        m = {
            "x_full": np.ascontiguousarray(x[b]),
            "x_resid": np.ascontiguousarray(
                x.reshape(T, H)[i * TS:(i + 1) * TS]),
            "wq": np.ascontiguousarray(
                wq[:, g * 4 * HD:(g + 1) * 4 * HD]).astype(bf16),
            "wk": np.ascontiguousarray(
                wk[:, g * 2 * HD:(g + 1) * 2 * HD]).astype(bf16),
            "wv": np.ascontiguousarray(
                wv[:, g * 2 * HD:(g + 1) * 2 * HD]).astype(bf16),
            "wo_pad": wo_pad.reshape(4 * P, H).astype(bf16),
            "cosT": cosT,
            "sinTs": sinT,
            "ln1w": ln1_t,
            "ln2w": ln2_t,
            "gate": gate,
            "w1": np.ascontiguousarray(w1[i]).astype(bf16),
            "w3": np.ascontiguousarray(w3[i]).astype(bf16),
            "w2": np.ascontiguousarray(w2[i]).astype(bf16),
            "myid": np.full((P, 1), float(i), f32),
        }
        in_maps.append(m)
    return in_maps


def kernel(**inputs) -> np.ndarray:
    nc = _get_nc()
    in_maps = _prep_inputs(inputs)
    res = run_bass_kernel_spmd(nc, in_maps, core_ids=list(range(8)))
    outs = [res.results[i]["out"] for i in range(8)]
    return np.concatenate(outs, axis=0).reshape(B, S, H).astype(np.float32)
